# revision 10
# baseline (speedup 1.0000x reference)
"""Trainium2 Bass kernel for nn_LlamaDLODecoderLayer (moe_routing).

Sharding: 8 cores = 4 batch rows x 2 query-halves. Each core processes the
512-query-token half of one row's K=1024 routed tokens; K/V are recomputed
for the full row on both cores of a pair (keeps the program SPMD-uniform,
no collectives). Host does routing gather/scatter, RoPE tables, weight
pre-tiling and fp8 cast; device does the full decoder layer with fp8
DoubleRow matmuls for all projections/MLP (2x PE throughput), bf16
attention scores/probs, fp32 softmax/residuals.

The kv rows are host-permuted so each core's query half is always kv rows
[0:512]; the q-side transposed activations are then just a slice of the
kv-side ones. Causal masks and RoPE tables are built for the permuted
order on the host. Power-of-2 fp8 scale factors are folded into the RoPE
tables, activation scales, and the residual-add multipliers, so descaling
costs no extra device ops.
"""

import sys

sys.path.insert(0, "/opt/trn_rl_repo")

import math
from contextlib import ExitStack

import ml_dtypes
import numpy as np

import concourse.bacc as bacc
import concourse.bass as bass
import concourse.mybir as mybir
import concourse.tile as tile
from concourse.bass_utils import run_bass_kernel_spmd
from concourse.masks import make_identity

B, S, H, NH, DH, DFF = 4, 8192, 2048, 16, 128, 8192
K = 1024
TQ = 512          # query tokens per core
TKV = 1024        # kv tokens per core (full row)
EPS = 1e-5
THETA = 10000.0
NEG = -1e9

F32 = mybir.dt.float32
BF16 = mybir.dt.bfloat16
FP8 = mybir.dt.float8e4
AF = mybir.ActivationFunctionType
DR = mybir.MatmulPerfMode.DoubleRow
MULT = mybir.AluOpType.mult
ADD = mybir.AluOpType.add
FP8NP = ml_dtypes.float8_e4m3

NKT = H // 128    # 16 k-tiles over H
NKP = NKT // 2    # 8 DoubleRow k-pairs over H
NFT = DFF // 128  # 64 f-tiles over DFF
NFP = NFT // 2    # 32 DoubleRow f-pairs over DFF

# fp8 scale factors (powers of two; folded into tables/activation scales)
AS = 8.0            # activation scale for xn / xn2
WSQ = 8192.0        # weight scale for Wq (includes 1/sqrt(DH) fold)
WS = 1024.0         # weight scale for Wk/Wv/Wo/Wg/Wu/Wd
OS = 16.0           # fp8 scale for attention output oT
HS2 = 16.0          # effective fp8 scale for MLP mid hT (via 2^-9 * WS*AS)

_COMPILED = {}


class St:
    """Shared build state."""
    pass


def _consts(st):
    nc, tc, ep = st.nc, st.tc, st.ep
    consts = ep(tc.tile_pool(name="consts", bufs=1))
    st.ident = consts.tile([128, 128], BF16)
    make_identity(nc, st.ident)
    st.ident8 = consts.tile([128, 128], FP8)
    make_identity(nc, st.ident8)
    st.cosq = consts.tile([DH, TQ], F32)
    st.sinq = consts.tile([DH, TQ], F32)
    st.cosk = consts.tile([DH, TKV], F32)
    st.sink = consts.tile([DH, TKV], F32)
    st.scale_sb = consts.tile([128, 1], F32)
    st.eps_sb = consts.tile([128, 1], F32)
    # eps/AS^2 so that 1/sqrt(mean/AS^2 + eps/AS^2) = AS/sqrt(mean+eps)
    nc.vector.memset(st.eps_sb[:], EPS / (AS * AS))
    nc.sync.dma_start(st.cosq[:], st.d["cosq"][:])
    nc.sync.dma_start(st.sinq[:], st.d["sinq"][:])
    nc.sync.dma_start(st.cosk[:], st.d["cosk"][:])
    nc.sync.dma_start(st.sink[:], st.d["sink"][:])
    nc.sync.dma_start(st.scale_sb[:], st.d["scale"][:])

    svals = ep(tc.tile_pool(name="svals", bufs=1))
    st.s_kv = svals.tile([128, 8], F32)
    st.s_2 = svals.tile([128, 4], F32)
    st.stmp = svals.tile([128, 32], F32)


def _inv_rms8(st, src_ap, dst_col, scratch, idx):
    """dst_col = AS / sqrt(mean(src^2) + EPS)  (AS fold via eps_sb/scale)."""
    nc = st.nc
    c0, c1 = 2 * idx, 2 * idx + 1
    nc.scalar.activation(scratch[:], src_ap, AF.Square,
                         accum_out=st.stmp[:, c0:c0 + 1])
    nc.scalar.activation(st.stmp[:, c1:c1 + 1], st.stmp[:, c0:c0 + 1], AF.Sqrt,
                         bias=st.eps_sb[:, 0:1], scale=1.0 / (H * AS * AS))
    nc.vector.reciprocal(dst_col, st.stmp[:, c1:c1 + 1])


def _transpose_fp8_into(st, pool, dst_ap, src_ap):
    """PE-transpose a [128,128] fp8 block into dst via PSUM (step-2 out)."""
    nc = st.nc
    tp = pool.tile([128, 128, 2], FP8, tag="tp8", name="tp8")
    nc.tensor.transpose(tp[:, :, 0:1], src_ap, st.ident8[:])
    nc.vector.tensor_copy(dst_ap, tp[:, :, 0:1])


def _transpose_into(st, pool, dst_ap, src_ap):
    """PE-transpose a [128,128] bf16 block into dst via PSUM."""
    nc = st.nc
    tp = pool.tile([128, 128], BF16, tag="tp", name="tp")
    nc.tensor.transpose(tp[:], src_ap, st.ident[:])
    nc.vector.tensor_copy(dst_ap, tp[:])


def _stage_norm1(st, s1):
    """xkv -> fp8 transposed xnT pairs [128, 2, TKV] x NKP."""
    nc, tc = st.nc, st.tc
    xload = s1.enter_context(tc.tile_pool(name="xload", bufs=2))
    sqscr = s1.enter_context(tc.tile_pool(name="sqscr", bufs=1))
    xn_pool = s1.enter_context(tc.tile_pool(name="xn", bufs=2))
    st.tpsum8 = s1.enter_context(tc.tile_pool(name="tpsum8", bufs=2,
                                              space="PSUM"))

    st.xnT = []
    for i in range(NKP):
        xi = st.xnT_pool.tile([128, 2, TKV], FP8, tag="xnT", name="xnTt")
        st.xnT.append(xi)

    for j in range(8):
        ld = xload.tile([128, H], BF16, tag="xload")
        nc.sync.dma_start(ld[:], st.d["xkv"][j * 128:(j + 1) * 128, :])
        scr = sqscr.tile([128, H], F32, tag="sq")
        _inv_rms8(st, ld[:], st.s_kv[:, j:j + 1], scr, j)
        xn_j = xn_pool.tile([128, H], FP8, tag="xn")
        nc.vector.tensor_scalar_mul(xn_j[:], ld[:], st.s_kv[:, j:j + 1])
        for i in range(NKT):
            _transpose_fp8_into(
                st, st.tpsum8,
                st.xnT[i // 2][:, i % 2, j * 128:(j + 1) * 128],
                xn_j[:, i * 128:(i + 1) * 128])


def _stage_v(st, s2):
    nc, tc = st.nc, st.tc
    wv_pool = s2.enter_context(tc.tile_pool(name="wv", bufs=8))
    vps = s2.enter_context(tc.tile_pool(name="vps", bufs=2, space="PSUM"))
    wv_sb = []
    for kp in range(NKP):
        wt = wv_pool.tile([128, 2, H], FP8, tag="wv")
        nc.sync.dma_start(wt[:], st.d["wv"][kp])
        wv_sb.append(wt)
    st.v_sb = []
    for j in range(8):
        vt = st.v_pool.tile([128, H], BF16, tag="v", name="vt")
        for n in range(4):
            ps = vps.tile([128, 512], F32, tag="vps")
            for kp in range(NKP):
                nc.tensor.matmul(ps[:],
                                 st.xnT[kp][:, :, j * 128:(j + 1) * 128],
                                 wv_sb[kp][:, :, n * 512:(n + 1) * 512],
                                 start=(kp == 0), stop=(kp == NKP - 1),
                                 perf_mode=DR)
            # descale 1/(AS*WS) = 2^-13
            nc.scalar.activation(vt[:, n * 512:(n + 1) * 512], ps[:], AF.Copy,
                                 scale=1.0 / (AS * WS))
        st.v_sb.append(vt)


def _rope(st, p, dst_ap, src_psum, cos_ap, sin_ap):
    nc = st.nc
    qf = p["ropes"].tile([128, 512], F32, tag="rpa", name="qf")
    nc.vector.tensor_copy(qf[:], src_psum)
    rot = p["ropes"].tile([128, 512], F32, tag="rpb", name="rot")
    nc.scalar.mul(rot[0:64, :], qf[64:128, :], -1.0)
    nc.scalar.copy(rot[64:128, :], qf[0:64, :])
    nc.vector.tensor_mul(rot[:], rot[:], sin_ap)
    nc.vector.tensor_mul(qf[:], qf[:], cos_ap)
    nc.vector.tensor_add(dst_ap, qf[:], rot[:])


def _attn_head(st, p, hh):
    nc = st.nc
    wq_sb = p["wqk"].tile([128, NKT, 128], FP8, tag="wq", name="wqt")
    nc.sync.dma_start(wq_sb[:], st.d["wq"][hh])
    wk_sb = p["wqk"].tile([128, NKT, 128], FP8, tag="wk", name="wkt")
    nc.sync.dma_start(wk_sb[:], st.d["wk"][hh])

    qr = p["qkr"].tile([128, TQ], BF16, tag="qr", name="qrt")
    kr = p["qkr"].tile([128, TKV], BF16, tag="kr", name="krt")

    qps = p["qkps"].tile([128, 512], F32, tag="qk", name="qpst")
    for kp in range(NKP):
        nc.tensor.matmul(qps[:], wq_sb[:, 2 * kp:2 * kp + 2, :],
                         st.xnT[kp][:, :, 0:TQ],
                         start=(kp == 0), stop=(kp == NKP - 1), perf_mode=DR)
    _rope(st, p, qr[:], qps[:], st.cosq[:], st.sinq[:])

    for half in range(2):
        kps = p["qkps"].tile([128, 512], F32, tag="qk", name="kpst")
        for kp in range(NKP):
            nc.tensor.matmul(kps[:], wk_sb[:, 2 * kp:2 * kp + 2, :],
                             st.xnT[kp][:, :, half * 512:(half + 1) * 512],
                             start=(kp == 0), stop=(kp == NKP - 1),
                             perf_mode=DR)
        _rope(st, p, kr[:, half * 512:(half + 1) * 512], kps[:],
              st.cosk[:, half * 512:(half + 1) * 512],
              st.sink[:, half * 512:(half + 1) * 512])

    aT = []
    for _ in range(8):
        aT_t = p["attnT"].tile([128, TQ], BF16, tag="attnT", name="aTt")
        aT.append(aT_t)
    for t in range(4):
        p0 = p["scps"].tile([128, 512], F32, tag="sc", name="p0t")
        nc.tensor.matmul(p0[:], qr[:, t * 128:(t + 1) * 128], kr[:, 0:512],
                         start=True, stop=True)
        p1 = p["scps"].tile([128, 512], F32, tag="sc", name="p1t")
        nc.tensor.matmul(p1[:], qr[:, t * 128:(t + 1) * 128], kr[:, 512:1024],
                         start=True, stop=True)
        sc = p["scsb"].tile([128, TKV], F32, tag="sc", name="sct")
        nc.vector.tensor_add(sc[:, 0:512], p0[:], st.mask_sb[t][:, 0:512])
        nc.vector.tensor_add(sc[:, 512:1024], p1[:], st.mask_sb[t][:, 512:1024])
        nm = p["smv"].tile([128, 3], F32, tag="smv", name="nmt")
        nc.vector.tensor_reduce(nm[:, 0:1], sc[:], axis=mybir.AxisListType.X,
                                op=mybir.AluOpType.max, negate=True)
        pr = p["scsb"].tile([128, TKV], BF16, tag="pr", name="prt")
        nc.scalar.activation(pr[:], sc[:], AF.Exp, bias=nm[:, 0:1],
                             accum_out=nm[:, 1:2])
        nc.vector.reciprocal(nm[:, 2:3], nm[:, 1:2])
        nc.vector.tensor_scalar_mul(pr[:], pr[:], nm[:, 2:3])
        for kk in range(8):
            _transpose_into(st, st.tpsum, aT[kk][:, t * 128:(t + 1) * 128],
                            pr[:, kk * 128:(kk + 1) * 128])

    ops = p["avps"].tile([128, TQ], F32, tag="av", name="opst")
    for kk in range(8):
        nc.tensor.matmul(ops[:], st.v_sb[kk][:, hh * 128:(hh + 1) * 128],
                         aT[kk][:], start=(kk == 0), stop=(kk == 7))
    # oT pairs for DoubleRow Wo: head hh -> pair hh//2, plane hh%2; x OS
    nc.scalar.activation(st.oT[hh // 2][:, hh % 2, :], ops[:], AF.Copy,
                         scale=OS)


def _stage_attn(st, s3):
    tc = st.tc
    p = {
        "wqk": s3.enter_context(tc.tile_pool(name="wqk", bufs=3)),
        "qkps": s3.enter_context(tc.tile_pool(name="qkps", bufs=3, space="PSUM")),
        "ropes": s3.enter_context(tc.tile_pool(name="ropes", bufs=4)),
        "qkr": s3.enter_context(tc.tile_pool(name="qkr", bufs=4)),
        "scps": s3.enter_context(tc.tile_pool(name="scps", bufs=2, space="PSUM")),
        "scsb": s3.enter_context(tc.tile_pool(name="scsb", bufs=3)),
        "smv": s3.enter_context(tc.tile_pool(name="smv", bufs=2)),
        "attnT": s3.enter_context(tc.tile_pool(name="attnT", bufs=12)),
        "avps": s3.enter_context(tc.tile_pool(name="avps", bufs=1, space="PSUM")),
    }
    st.tpsum = s3.enter_context(tc.tile_pool(name="tpsum", bufs=2,
                                             space="PSUM"))
    st.oT = []
    for pp in range(NH // 2):
        oT_t = st.oT_pool.tile([128, 2, TQ], FP8, tag="oT", name="oTt")
        st.oT.append(oT_t)
    for hh in range(NH):
        _attn_head(st, p, hh)


def _stage_wo(st, s4):
    nc, tc = st.nc, st.tc
    xn2_sb = []
    st.xn2_pool = s4.enter_context(tc.tile_pool(name="xn2", bufs=4))
    with ExitStack() as s4w:
        xq_pool = s4w.enter_context(tc.tile_pool(name="xq2", bufs=4))
        wo_pool = s4w.enter_context(tc.tile_pool(name="wo", bufs=8))
        sq2 = s4w.enter_context(tc.tile_pool(name="sq2", bufs=1))
        wops = s4w.enter_context(tc.tile_pool(name="wops", bufs=8, space="PSUM"))
        xq_sb = []
        for t in range(4):
            xt = xq_pool.tile([128, H], F32, tag="xq2", name="xqt")
            nc.sync.dma_start(xt[:], st.d["xq"][t * 128:(t + 1) * 128, :])
            xq_sb.append(xt)
        wo_sb = []
        for kp in range(NKP):
            wt = wo_pool.tile([128, 2, H], FP8, tag="wo", name="wot")
            nc.sync.dma_start(wt[:], st.d["wo"][kp])
            wo_sb.append(wt)
        for tp2 in range(2):
            ps = []
            for _a in range(2):
                row = []
                for _b in range(4):
                    pst = wops.tile([128, 512], F32, tag="wops", name="wopst")
                    row.append(pst)
                ps.append(row)
            for kp in range(NKP):
                for t2 in range(2):
                    t = tp2 * 2 + t2
                    for n in range(4):
                        nc.tensor.matmul(ps[t2][n][:],
                                         st.oT[kp][:, :, t * 128:(t + 1) * 128],
                                         wo_sb[kp][:, :, n * 512:(n + 1) * 512],
                                         start=(kp == 0), stop=(kp == NKP - 1),
                                         perf_mode=DR)
            for t2 in range(2):
                t = tp2 * 2 + t2
                hs_t = st.hs_pool.tile([128, H], F32, tag="hs", name="hst")
                for n in range(4):
                    # hs = ps/(OS*WS) + xq   (one fused DVE op per slice)
                    nc.vector.scalar_tensor_tensor(
                        hs_t[:, n * 512:(n + 1) * 512], ps[t2][n][:],
                        1.0 / (OS * WS),
                        xq_sb[t][:, n * 512:(n + 1) * 512], MULT, ADD)
                scr = sq2.tile([128, H], F32, tag="sq2", name="scrt")
                _inv_rms8(st, hs_t[:], st.s_2[:, t:t + 1], scr, 12 + t)
                xn2_t = st.xn2_pool.tile([128, H], FP8, tag="xn2", name="xn2t")
                nc.vector.tensor_scalar_mul(xn2_t[:], hs_t[:], st.s_2[:, t:t + 1])
                st.hs_sb.append(hs_t)
                xn2_sb.append(xn2_t)
    # transposes (after wops PSUM freed)
    tp2pool = s4.enter_context(tc.tile_pool(name="tpsum2", bufs=2, space="PSUM"))
    st.xn2T = []
    for kp in range(NKP):
        xi = st.xn2T_pool.tile([128, 2, TQ], FP8, tag="xn2T", name="xn2Tt")
        st.xn2T.append(xi)
    for i in range(NKT):
        for t in range(4):
            _transpose_fp8_into(
                st, tp2pool,
                st.xn2T[i // 2][:, i % 2, t * 128:(t + 1) * 128],
                xn2_sb[t][:, i * 128:(i + 1) * 128])


def _stage_mlp_gu(st, s5):
    nc, tc = st.nc, st.tc
    wgu_pool = s5.enter_context(tc.tile_pool(name="wgu", bufs=6))
    gps_pool = s5.enter_context(tc.tile_pool(name="gps", bufs=2, space="PSUM"))
    ups_pool = s5.enter_context(tc.tile_pool(name="ups", bufs=2, space="PSUM"))
    gsc = s5.enter_context(tc.tile_pool(name="gsc", bufs=2))
    st.hT = []
    for fp in range(NFP):
        hT_t = st.hT_pool.tile([128, 2, TQ], FP8, tag="hT", name="hTt")
        st.hT.append(hT_t)
    for f in range(NFT):
        wg_sb = wgu_pool.tile([128, NKT, 128], FP8, tag="wg", name="wgt")
        nc.sync.dma_start(wg_sb[:], st.d["wg"][f])
        wu_sb = wgu_pool.tile([128, NKT, 128], FP8, tag="wu", name="wut")
        nc.sync.dma_start(wu_sb[:], st.d["wu"][f])
        gps = gps_pool.tile([128, TQ], F32, tag="g", name="gpst")
        ups = ups_pool.tile([128, TQ], F32, tag="u", name="upst")
        for kp in range(NKP):
            nc.tensor.matmul(gps[:], wg_sb[:, 2 * kp:2 * kp + 2, :],
                             st.xn2T[kp][:], start=(kp == 0),
                             stop=(kp == NKP - 1), perf_mode=DR)
        for kp in range(NKP):
            nc.tensor.matmul(ups[:], wu_sb[:, 2 * kp:2 * kp + 2, :],
                             st.xn2T[kp][:], start=(kp == 0),
                             stop=(kp == NKP - 1), perf_mode=DR)
        gs = gsc.tile([128, TQ], F32, tag="gs", name="gst")
        # gps = AS*WS*g -> silu(g) exact
        nc.scalar.activation(gs[:], gps[:], AF.Silu, scale=1.0 / (AS * WS))
        # hT = (gs * HS2/(AS*WS)) * ups = HS2 * silu(g) * u   in fp8
        nc.vector.scalar_tensor_tensor(
            st.hT[f // 2][:, f % 2, :], gs[:], HS2 / (AS * WS), ups[:],
            MULT, MULT)


def _stage_down(st, s6):
    nc, tc = st.nc, st.tc
    wd_pool = s6.enter_context(tc.tile_pool(name="wd", bufs=3))
    dnps = s6.enter_context(tc.tile_pool(name="dnps", bufs=8, space="PSUM"))
    for nh in range(2):
        ps = []
        for _a in range(4):
            row = []
            for _b in range(2):
                pst = dnps.tile([128, 512], F32, tag="dn", name="dnt")
                row.append(pst)
            ps.append(row)
        for fp in range(NFP):
            wd_sb = wd_pool.tile([128, 2, 1024], FP8, tag="wd", name="wdt")
            nc.sync.dma_start(wd_sb[:],
                              st.d["wd"][fp][:, :, nh * 1024:(nh + 1) * 1024])
            for t in range(4):
                for n2 in range(2):
                    nc.tensor.matmul(ps[t][n2][:],
                                     st.hT[fp][:, :, t * 128:(t + 1) * 128],
                                     wd_sb[:, :, n2 * 512:(n2 + 1) * 512],
                                     start=(fp == 0), stop=(fp == NFP - 1),
                                     perf_mode=DR)
        fin = s6.enter_context(tc.tile_pool(name=f"fin{nh}", bufs=2))
        for t in range(4):
            for n2 in range(2):
                col = nh * 1024 + n2 * 512
                ft = fin.tile([128, 512], F32, tag="fin", name="ft")
                # ft = ps * (topk_scale/(HS2*WS)) + hs   (scale pre-folded)
                nc.vector.scalar_tensor_tensor(
                    ft[:], ps[t][n2][:], st.scale_sb[:, 0:1],
                    st.hs_sb[t][:, col:col + 512], MULT, ADD)
                nc.sync.dma_start(st.d["out"][t * 128:(t + 1) * 128,
                                              col:col + 512], ft[:])


def _build(reps=1):
    nc = bacc.Bacc()
    st = St()
    st.nc = nc
    d = {}
    d["xq"] = nc.dram_tensor("xq", [TQ, H], F32, kind="ExternalInput")
    d["xkv"] = nc.dram_tensor("xkv", [TKV, H], BF16, kind="ExternalInput")
    d["cosq"] = nc.dram_tensor("cosq", [DH, TQ], F32, kind="ExternalInput")
    d["sinq"] = nc.dram_tensor("sinq", [DH, TQ], F32, kind="ExternalInput")
    d["cosk"] = nc.dram_tensor("cosk", [DH, TKV], F32, kind="ExternalInput")
    d["sink"] = nc.dram_tensor("sink", [DH, TKV], F32, kind="ExternalInput")
    d["mask"] = nc.dram_tensor("mask", [TQ, TKV], F32, kind="ExternalInput")
    d["scale"] = nc.dram_tensor("scale", [128, 1], F32, kind="ExternalInput")
    d["wq"] = nc.dram_tensor("wq", [NH, 128, NKT, 128], FP8, kind="ExternalInput")
    d["wk"] = nc.dram_tensor("wk", [NH, 128, NKT, 128], FP8, kind="ExternalInput")
    d["wv"] = nc.dram_tensor("wv", [NKP, 128, 2, H], FP8, kind="ExternalInput")
    d["wo"] = nc.dram_tensor("wo", [NKP, 128, 2, H], FP8, kind="ExternalInput")
    d["wg"] = nc.dram_tensor("wg", [NFT, 128, NKT, 128], FP8, kind="ExternalInput")
    d["wu"] = nc.dram_tensor("wu", [NFT, 128, NKT, 128], FP8, kind="ExternalInput")
    d["wd"] = nc.dram_tensor("wd", [NFP, 128, 2, H], FP8, kind="ExternalInput")
    d["out"] = nc.dram_tensor("out", [TQ, H], F32, kind="ExternalOutput")
    st.d = d

    with tile.TileContext(nc) as tc, ExitStack() as ctx:
        st.tc = tc
        st.ep = ctx.enter_context
        _consts(st)
        for _rep in range(reps):
            _layer_body(st)

    nc.compile()
    return nc


def _layer_body(st):
    nc, tc = st.nc, st.tc
    d = st.d
    with ExitStack() as body:
        st.oT_pool = body.enter_context(tc.tile_pool(name="oT", bufs=8))
        with ExitStack() as s123:
            e = s123.enter_context
            mask_pool = e(tc.tile_pool(name="maskp", bufs=4))
            st.mask_sb = []
            for t in range(4):
                mt = mask_pool.tile([128, TKV], F32, tag="mask", name="mt")
                nc.sync.dma_start(mt[:], d["mask"][t * 128:(t + 1) * 128, :])
                st.mask_sb.append(mt)
            st.xnT_pool = e(tc.tile_pool(name="xnT", bufs=8))
            st.v_pool = e(tc.tile_pool(name="vsb", bufs=8))
            with ExitStack() as s1:
                _stage_norm1(st, s1)
            with ExitStack() as s2:
                _stage_v(st, s2)
            with ExitStack() as s3:
                _stage_attn(st, s3)
        with ExitStack() as s4567:
            e = s4567.enter_context
            st.hs_pool = e(tc.tile_pool(name="hs", bufs=4))
            st.xn2T_pool = e(tc.tile_pool(name="xn2T", bufs=8))
            st.hs_sb = []
            with ExitStack() as s4:
                _stage_wo(st, s4)
            with ExitStack() as s56:
                st.hT_pool = s56.enter_context(tc.tile_pool(name="hT", bufs=32))
                with ExitStack() as s5:
                    _stage_mlp_gu(st, s5)
                with ExitStack() as s6:
                    _stage_down(st, s6)


def _fp8w(w, scale):
    ws = w * scale
    amax = np.abs(ws).max()
    assert amax < 240.0, f"fp8 overflow: {amax}"
    return ws.astype(FP8NP)


def _prep_host(hidden_states, position_ids, topk_mask, topk_scores,
               Wq, Wk, Wv, Wo, Wgate, Wup, Wdown, ln1_w, ln2_w):
    bf16 = ml_dtypes.bfloat16
    order = np.argsort(np.where(topk_mask, 0, 1).astype(np.int32),
                       axis=1, kind="stable")
    topk_idx = order[:, :K]                                    # [B,K]
    bidx = np.arange(B)[:, None]
    x = np.ascontiguousarray(hidden_states[bidx, topk_idx])    # [B,K,H] f32
    pos = position_ids[bidx, topk_idx].astype(np.float32)      # [B,K]

    inv_freq = (1.0 / (THETA ** (np.arange(0, DH, 2, dtype=np.float32) / DH))
                ).astype(np.float32)
    freqs = pos[..., None] * inv_freq                          # [B,K,64]
    emb = np.concatenate([freqs, freqs], axis=-1)              # [B,K,128]
    cosT = np.ascontiguousarray(np.cos(emb).astype(np.float32).transpose(0, 2, 1))
    sinT = np.ascontiguousarray(np.sin(emb).astype(np.float32).transpose(0, 2, 1))

    l1 = ln1_w.astype(np.float32)[:, None]
    l2 = ln2_w.astype(np.float32)[:, None]
    wq_t = _fp8w(np.ascontiguousarray(
        (Wq * l1 / math.sqrt(DH)).reshape(16, 128, 16, 128)
        .transpose(2, 1, 0, 3)), WSQ)                          # [16,128,16,128]
    wk_t = _fp8w(np.ascontiguousarray(
        (Wk * l1).reshape(16, 128, 16, 128).transpose(2, 1, 0, 3)), WS)
    # DoubleRow pair layouts [NKP,128,2,H] over the contraction dim
    wv_t = _fp8w(np.ascontiguousarray(
        (Wv * l1).reshape(NKP, 2, 128, H).transpose(0, 2, 1, 3)), WS)
    wo_t = _fp8w(np.ascontiguousarray(
        Wo.reshape(NKP, 2, 128, H).transpose(0, 2, 1, 3)), WS)
    wg_t = _fp8w(np.ascontiguousarray(
        (Wgate * l2).reshape(16, 128, 64, 128).transpose(2, 1, 0, 3)), WS)
    wu_t = _fp8w(np.ascontiguousarray(
        (Wup * l2).reshape(16, 128, 64, 128).transpose(2, 1, 0, 3)), WS)
    wd_t = _fp8w(np.ascontiguousarray(
        Wdown.reshape(NFP, 2, 128, H).transpose(0, 2, 1, 3)), WS)

    qi = np.arange(TQ, dtype=np.int64)

    in_maps = []
    for c in range(8):
        b, h = c // 2, c % 2
        q0 = h * TQ
        # permuted kv order: [q-half tokens, remaining tokens]
        perm = np.concatenate([np.arange(q0, q0 + TQ),
                               np.arange(0, q0),
                               np.arange(q0 + TQ, TKV)]).astype(np.int64)
        orig_q = q0 + qi                                       # original q idx
        orig_k = perm                                          # original kv idx
        mask = np.where(orig_k[None, :] <= orig_q[:, None],
                        np.float32(0.0), np.float32(NEG)).astype(np.float32)
        scale_val = np.float32(
            (0.5 * 1.0 + (topk_scores[b] - 0.5) * 1.0) / (HS2 * WS))
        in_maps.append({
            "xq": np.ascontiguousarray(x[b, q0:q0 + TQ]).astype(np.float32),
            "xkv": np.ascontiguousarray(x[b][perm]).astype(bf16),
            "cosq": np.ascontiguousarray(cosT[b][:, q0:q0 + TQ]) / (AS * WSQ),
            "sinq": np.ascontiguousarray(sinT[b][:, q0:q0 + TQ]) / (AS * WSQ),
            "cosk": np.ascontiguousarray(cosT[b][:, perm]) / (AS * WS),
            "sink": np.ascontiguousarray(sinT[b][:, perm]) / (AS * WS),
            "mask": mask,
            "scale": np.full((128, 1), scale_val, dtype=np.float32),
            "wq": wq_t, "wk": wk_t, "wv": wv_t, "wo": wo_t,
            "wg": wg_t, "wu": wu_t, "wd": wd_t,
        })
    return in_maps, topk_idx, x


def kernel(hidden_states, position_ids, topk_mask, topk_scores, topk_k,
           Wq, Wk, Wv, Wo, Wgate, Wup, Wdown, ln1_w, ln2_w,
           _want_trace=False):
    assert int(topk_k) == K
    hidden_states = np.asarray(hidden_states, dtype=np.float32)
    in_maps, topk_idx, _ = _prep_host(
        hidden_states, np.asarray(position_ids),
        np.asarray(topk_mask), np.asarray(topk_scores, dtype=np.float32),
        np.asarray(Wq, dtype=np.float32), np.asarray(Wk, dtype=np.float32),
        np.asarray(Wv, dtype=np.float32), np.asarray(Wo, dtype=np.float32),
        np.asarray(Wgate, dtype=np.float32), np.asarray(Wup, dtype=np.float32),
        np.asarray(Wdown, dtype=np.float32),
        np.asarray(ln1_w, dtype=np.float32), np.asarray(ln2_w, dtype=np.float32))

    if "nc" not in _COMPILED:
        _COMPILED["nc"] = _build()
    nc = _COMPILED["nc"]

    res = run_bass_kernel_spmd(nc, in_maps, list(range(8)), trace=_want_trace)
    kernel.last_exec_time_ns = res.exec_time_ns
    kernel.last_trace = res.instructions_and_trace

    out = hidden_states.copy()
    for c in range(8):
        b, h = c // 2, c % 2
        q0 = h * TQ
        out[b, topk_idx[b, q0:q0 + TQ]] = res.results[c]["out"]
    return out


kernel.last_exec_time_ns = None
kernel.last_trace = None
